# revision 39
# baseline (speedup 1.0000x reference)
"""Multi-head self-attention with RoPE on 8 Trainium2 NeuronCores.

Sharding: core c = batch*2 + head_group. Each core computes 8 of the 16
heads for one batch element end-to-end (QKV projection, RoPE, attention,
out-projection partial). Host sums the two head-group partials per batch
and applies the (linear) bias corrections.

All matmuls run in bf16 with fp32 PSUM accumulation. Softmax skips the
max-subtraction (scores for this problem are bounded by a few units, so
exp() is safe in fp32) and folds the row-sum into the P@V matmul via a
ones-column on V; normalization happens on the tiny [64, t] output.

Schedule: one flat software-pipelined stream of 256 attention granules
(score-pair matmul -> exp on ScalarE -> two P@V matmuls). Scores run one
granule ahead of AV so the PE never stalls on the exp; projection / V /
out-projection work is chopped into generator "filler" chunks pumped
into the exp-wait slots, with deadline-keyed draining (the per-engine
instruction queues are strict FIFO, so emission order is the schedule).
RoPE elementwise runs in bf16 split across GpSimd and DVE; the RoPE
pair-permute uses a PE matmul on the startup-critical K chains and
partition-moving SBUF->SBUF DMAs in steady state. AV psum is evacuated
to SBUF immediately (frees the 2 accumulator banks) and the softmax
denominators are gathered via DMA into a [32,32] tile for one batched
reciprocal per head-pair. Input loads are issued from the Scalar-engine
DGE queue so small latency-critical DMAs don't queue behind them; dummy
matmuls at t=0 keep the PE clock warm (HAM) while inputs stream in; y is
stored as bf16 (host accumulates partials in fp32); the last out-proj
tile is split so only its final head-pair contribution runs in the tail.
PSUM: scores [128,1024]x2 + AV [65,512]x2 + work [128,512]x2 = 8 banks.
"""

import numpy as np
import ml_dtypes

# ---------------------------------------------------------------------------
# Workaround: this walrus build rejects >1 sem-wait on a CTRL-only (Drain)
# instruction. TileContext's tail drain carries one wait per outstanding
# logical proc; split them across a chain of single-wait drains.
# ---------------------------------------------------------------------------
_PATCHED = False


def _split_waits_json(raw: bytes) -> bytes:
    """Split instructions carrying >1 sem-wait into single-wait NoOp
    carriers followed by the original instruction (this walrus build
    allows at most one sync-wait per instruction)."""
    import json

    m = json.loads(raw)

    def fix_block(bb):
        insts = bb.get("instructions")
        if not isinstance(insts, list):
            return
        out = []
        for inst in insts:
            si = inst.get("sync_info") if isinstance(inst, dict) else None
            waits = si.get("on_wait") if si else None
            if waits and len(waits) > 1:
                for k, w in enumerate(waits[:-1]):
                    out.append({
                        "debug": inst.get("debug"),
                        "engine": inst["engine"],
                        "ins": [], "outs": [],
                        "name": f'{inst["name"]}_wc{k}',
                        "opcode": "NoOp",
                        "sync_info": {"on_update": [], "on_wait": [w]},
                        "text_hint": "waitsplit",
                    })
                si["on_wait"] = [waits[-1]]
            out.append(inst)
        bb["instructions"] = out

    def walk(obj):
        if isinstance(obj, dict):
            if "instructions" in obj:
                fix_block(obj)
            for v in obj.values():
                walk(v)
        elif isinstance(obj, list):
            for v in obj:
                walk(v)

    walk(m)
    return json.dumps(m).encode()


def _apply_tile_patch():
    global _PATCHED
    if _PATCHED:
        return
    import concourse.bass as bass

    orig = bass.Bass.to_json_bytes

    def to_json_bytes_split(self, *a, **kw):
        return _split_waits_json(orig(self, *a, **kw))

    bass.Bass.to_json_bytes = to_json_bytes_split
    _PATCHED = True


# ---------------------------------------------------------------------------
# Problem dims (hardcoded for the full problem; parameterized for testing)
# ---------------------------------------------------------------------------
class Cfg:
    def __init__(self, T=2048, CIN=1024, JH=512, CO=1024, D=64):
        self.T, self.CIN, self.JH, self.CO, self.D = T, CIN, JH, CO, D
        self.H = JH // D            # heads per core
        self.NCC = CIN // 128       # contraction chunks
        self.NJ = JH // 128         # q/k row tiles
        self.NT = T // 128          # t partition tiles (= s chunks)
        self.TC = 512               # matmul moving-dim chunk
        self.NTC = T // self.TC
        assert JH % 128 == 0 and CIN % 128 == 0
        assert D == 64, "RoPE layout assumes D=64 (pairs at +-32 partitions)"
        assert self.NJ == 4 and self.NTC == 4, "v2 schedule assumes 4x4"


def rope_tables(cfg, dtype=np.float32):
    """cos/sin tables laid out for the [j-within-tile, t] orientation.

    Partition p of a q/k row-tile holds head-channel d = p % 64; the RoPE
    pair of d is d^32 within the same 64-block. sin is sign-baked:
    negative for the first half of each head, positive for the second.
    """
    half = cfg.D // 2
    theta = (10000.0 ** (-np.arange(half, dtype=np.float32) / half)).astype(np.float32)
    t = np.arange(cfg.T, dtype=np.float32)
    freqs = t[None, :] * theta[:, None]          # (32, T) fp32, matches reference
    cos32, sin32 = np.cos(freqs), np.sin(freqs)
    cos = np.tile(cos32, (4, 1))                 # (128, T)
    sgn = np.where((np.arange(128) % 64) < 32, -1.0, 1.0).astype(np.float32)
    sin = np.tile(sin32, (4, 1)) * sgn[:, None]
    return cos.astype(dtype), sin.astype(dtype)


def perm_matrix():
    """[128,128] permutation: out[p] = in[sigma(p)], sigma(p) = p^32 in 64-blocks."""
    m = np.zeros((128, 128), dtype=np.float32)
    k = np.arange(128)
    sigma = (k // 64) * 64 + (k + 32) % 64
    m[k, sigma] = 1.0
    return m.astype(ml_dtypes.bfloat16)


# ---------------------------------------------------------------------------
# Bass program
# ---------------------------------------------------------------------------
def build_nc(cfg, with_qk_bias=False):
    _apply_tile_patch()
    import concourse.bass as bass
    import concourse.tile as tile
    from concourse import mybir
    import contextlib

    from concourse.alu_op_type import AluOpType as AluOp

    f32 = mybir.dt.float32
    bf16 = mybir.dt.bfloat16
    i16 = mybir.dt.int16
    # Schraudolph exp in bf16-bit-space: bits = A*z + B, z = score.
    # A folds the 1/sqrt(D) softmax scale; B centers the sawtooth error
    # (sigma_ln = -0.0397 -> ratio mean 1.0, max +-2.3%).
    SCHRAUD_MOD = 0
    SCHRAUD_A = float(128.0 * np.log2(np.e) / np.sqrt(64.0))
    SCHRAUD_B = float(128.0 * (127.0 - 0.0397 / np.log(2.0)))
    nc = bass.Bass()

    xT = nc.dram_tensor("xT", (cfg.CIN, cfg.T), bf16, kind="ExternalInput")
    wqT = nc.dram_tensor("wqT", (cfg.CIN, cfg.JH), bf16, kind="ExternalInput")
    wkT = nc.dram_tensor("wkT", (cfg.CIN, cfg.JH), bf16, kind="ExternalInput")
    wvT = nc.dram_tensor("wvT", (cfg.CIN, cfg.JH), bf16, kind="ExternalInput")
    woT = nc.dram_tensor("woT", (cfg.JH, cfg.CO), bf16, kind="ExternalInput")
    cosT = nc.dram_tensor("cosT", (128, cfg.T), bf16, kind="ExternalInput")
    sinT = nc.dram_tensor("sinT", (128, cfg.T), bf16, kind="ExternalInput")
    permM = nc.dram_tensor("permM", (128, 128), bf16, kind="ExternalInput")
    if with_qk_bias:
        bqD = nc.dram_tensor("bq", (cfg.NJ, 128), f32, kind="ExternalInput")
        bkD = nc.dram_tensor("bk", (cfg.NJ, 128), f32, kind="ExternalInput")
    y = nc.dram_tensor("y", (cfg.T, cfg.CO), bf16, kind="ExternalOutput")

    NCC, NJ, NT, TC, NTC, H, D = (
        cfg.NCC, cfg.NJ, cfg.NT, cfg.TC, cfg.NTC, cfg.H, cfg.D)

    with tile.TileContext(nc) as tc:
        with contextlib.ExitStack() as ctx:
            consts = ctx.enter_context(tc.tile_pool(name="consts", bufs=1))
            slabs = ctx.enter_context(tc.tile_pool(name="slabs", bufs=1))
            evac = ctx.enter_context(tc.tile_pool(name="evac", bufs=3))
            ropetmp = ctx.enter_context(tc.tile_pool(name="ropetmp", bufs=4))
            ppool = ctx.enter_context(tc.tile_pool(name="ppool", bufs=5))
            ypool = ctx.enter_context(tc.tile_pool(name="ypool", bufs=2))
            avsb = ctx.enter_context(tc.tile_pool(name="avsb", bufs=2))
            rbpool = ctx.enter_context(tc.tile_pool(name="rbpool", bufs=2))
            rcp = ctx.enter_context(tc.tile_pool(name="rcp", bufs=4))
            ypartp = ctx.enter_context(tc.tile_pool(name="ypartp", bufs=8))
            rdram = ctx.enter_context(tc.tile_pool(name="rdram", bufs=4, space="DRAM"))

            # ---- constants / weights / activations (DMA order = use order) ----
            cos_sb = consts.tile([128, cfg.T], bf16)
            sin_sb = consts.tile([128, cfg.T], bf16)
            perm_sb = consts.tile([128, 128], bf16)

            w_sbs = {}
            for name in ("q", "k", "v"):
                w_sbs[name] = slabs.tile([128, NCC, cfg.JH], bf16, tag=f"w{name}",
                                         name=f"w{name}_sb")
            x_sb = slabs.tile([128, NCC, cfg.T], bf16)
            xT_v = xT[:, :].rearrange("(cc p) t -> p cc t", p=128)
            # interleave the K-weight and first-x-chunk DMAs per cc chunk so
            # the first projection matmul can start early.
            for cc in range(NCC):
                nc.scalar.dma_start(
                    out=w_sbs["k"][:, cc, :],
                    in_=wkT[cc * 128:(cc + 1) * 128, :])
                nc.scalar.dma_start(out=x_sb[:, cc, 0:TC],
                                    in_=xT_v[:, cc, 0:TC])
            nc.scalar.dma_start(out=perm_sb, in_=permM[:, :])
            # only the first t-chunk of the trig tables gates the startup
            # RoPE chains; the rest can land after wq/wv.
            nc.scalar.dma_start(out=cos_sb[:, 0:TC], in_=cosT[:, 0:TC])
            nc.scalar.dma_start(out=sin_sb[:, 0:TC], in_=sinT[:, 0:TC])
            nc.scalar.dma_start(out=w_sbs["q"],
                              in_=wqT[:, :].rearrange("(cc p) j -> p cc j", p=128))
            nc.scalar.dma_start(out=cos_sb[:, TC:], in_=cosT[:, TC:])
            nc.scalar.dma_start(out=sin_sb[:, TC:], in_=sinT[:, TC:])
            nc.scalar.dma_start(out=w_sbs["v"],
                              in_=wvT[:, :].rearrange("(cc p) j -> p cc j", p=128))
            for tq in range(1, NTC):
                tql = slice(tq * TC, (tq + 1) * TC)
                nc.scalar.dma_start(out=x_sb[:, :, tql], in_=xT_v[:, :, tql])
            wo_sb = slabs.tile([128, NJ, cfg.CO], bf16)
            nc.scalar.dma_start(out=wo_sb, in_=woT[:, :].rearrange("(jc p) o -> p jc o", p=128))
            if with_qk_bias:
                bq_sb = consts.tile([128, NJ], f32)
                bk_sb = consts.tile([128, NJ], f32)
                nc.sync.dma_start(out=bq_sb, in_=bqD[:, :].rearrange("j p -> p j"))
                nc.sync.dma_start(out=bk_sb, in_=bkD[:, :].rearrange("j p -> p j"))

            qr_sb = slabs.tile([128, NJ, cfg.T], bf16, tag="qr")
            kr_sb = slabs.tile([128, NJ, cfg.T], bf16, tag="kr")
            v_sb = slabs.tile([128, NT, H, D + 1], bf16, tag="vaug")
            ao_sb = slabs.tile([128, NJ, cfg.T], bf16, tag="ao")
            # ones column for the rowsum trick
            nc.gpsimd.memset(v_sb[:, :, :, D:D + 1], 1.0)

            # PSUM tags (per-tag rings): scores 2x2 banks, AV accumulators
            # 2x1, work (proj/vproj/outproj) 2x1 -> 8 banks total.
            psum = ctx.enter_context(tc.tile_pool(name="psum", bufs=2, space="PSUM"))

            # ---------------- filler machinery ----------------
            # PE instruction streams are FIFO: an instruction whose inputs
            # are not ready head-blocks everything behind it. So projection /
            # V / out-proj work is emitted in small generator chunks, pumped
            # into the exp-wait gaps of the attention loop, with explicit
            # deadline draining for correct ordering.
            import collections as _coll
            fillers = _coll.deque()          # (key, generator)
            seq = [0]

            def enqueue(gen):
                seq[0] += 1
                fillers.append((seq[0], gen))
                return seq[0]

            def pump(n=3):
                done = 0
                while fillers and done < n:
                    try:
                        next(fillers[0][1])
                        done += 1
                    except StopIteration:
                        fillers.popleft()

            def drain_key(key):
                while fillers and fillers[0][0] <= key:
                    try:
                        next(fillers[0][1])
                    except StopIteration:
                        fillers.popleft()

            def drain_all():
                drain_key(seq[0])

            def run_gen(gen):
                for _ in gen:
                    pass

            def gen_proj(name, dst, jt, tcc, mm_perm=False):
                """q/k projection + RoPE for one [128, TC] chunk, 4 chunks."""
                tsl = slice(tcc * TC, (tcc + 1) * TC)
                w = w_sbs[name]
                ps = psum.tile([128, TC], f32, tag="work",
                               name=f"proj_{name}{jt}_{tcc}")
                for cc in range(NCC // 2):
                    nc.tensor.matmul(
                        ps, lhsT=w[:, cc, jt * 128:(jt + 1) * 128],
                        rhs=x_sb[:, cc, tsl],
                        start=(cc == 0), stop=False)
                yield
                for cc in range(NCC // 2, NCC):
                    nc.tensor.matmul(
                        ps, lhsT=w[:, cc, jt * 128:(jt + 1) * 128],
                        rhs=x_sb[:, cc, tsl],
                        start=False, stop=(cc == NCC - 1))
                if with_qk_bias:
                    b = bq_sb if name == "q" else bk_sb
                    nc.vector.tensor_scalar_add(ps, ps, b[:, jt:jt + 1])
                qb = evac.tile([128, TC], bf16, tag="qb")
                nc.vector.tensor_copy(qb, ps)
                yield
                # RoPE pair permute (p <-> p^32 within 64-blocks): via a PE
                # matmul during the latency-critical startup chains, via four
                # partition-moving SBUF->SBUF DMAs in steady state.
                ppb = evac.tile([128, TC], bf16, tag="ppb")
                if mm_perm:
                    pp = psum.tile([128, TC], f32, tag="work",
                                   name=f"perm_{name}{jt}_{tcc}")
                    nc.tensor.matmul(pp, lhsT=perm_sb, rhs=qb)
                    nc.vector.tensor_copy(ppb, pp)
                else:
                    for blk in range(2):
                        b0 = 64 * blk
                        nc.sync.dma_start(out=ppb[b0:b0 + 32, :],
                                          in_=qb[b0 + 32:b0 + 64, :])
                        nc.sync.dma_start(out=ppb[b0 + 32:b0 + 64, :],
                                          in_=qb[b0:b0 + 32, :])
                t1 = ropetmp.tile([128, TC], bf16, tag="t1")
                nc.gpsimd.tensor_mul(t1, qb, cos_sb[:, tsl])
                yield
                t2 = ropetmp.tile([128, TC], bf16, tag="t2")
                nc.vector.tensor_mul(t2, ppb, sin_sb[:, tsl])
                nc.vector.tensor_add(dst[:, jt, tsl], t1, t2)

            def gen_vproj(tt):
                """V projection for t-tile tt, 2 chunks."""
                ps = psum.tile([128, TC], f32, tag="work", name=f"vproj_{tt}")
                for cc in range(NCC // 2):
                    nc.tensor.matmul(
                        ps, lhsT=x_sb[:, cc, tt * 128:(tt + 1) * 128],
                        rhs=w_sbs["v"][:, cc, :],
                        start=(cc == 0), stop=False)
                yield
                for cc in range(NCC // 2, NCC):
                    nc.tensor.matmul(
                        ps, lhsT=x_sb[:, cc, tt * 128:(tt + 1) * 128],
                        rhs=w_sbs["v"][:, cc, :],
                        start=False, stop=(cc == NCC - 1))
                nc.vector.tensor_copy(
                    v_sb[:, tt, :, 0:D],
                    ps[:, :].rearrange("p (h d) -> p h d", h=H))

            def gen_outproj(tt, u):
                """Half of the out-projection for t-tile tt, 2 chunks."""
                ps = psum.tile([128, TC], f32, tag="work",
                               name=f"yps_{tt}_{u}")
                for jc in range(2):
                    nc.tensor.matmul(
                        ps, lhsT=ao_sb[:, jc, tt * 128:(tt + 1) * 128],
                        rhs=wo_sb[:, jc, u * TC:(u + 1) * TC],
                        start=(jc == 0), stop=False)
                yield
                for jc in range(2, NJ):
                    nc.tensor.matmul(
                        ps, lhsT=ao_sb[:, jc, tt * 128:(tt + 1) * 128],
                        rhs=wo_sb[:, jc, u * TC:(u + 1) * TC],
                        start=False, stop=(jc == NJ - 1))
                yb = ypool.tile([128, TC], bf16, tag="yb", name=f"yb_{tt}_{u}")
                nc.vector.tensor_copy(yb, ps)
                nc.sync.dma_start(
                    out=y[tt * 128:(tt + 1) * 128, u * TC:(u + 1) * TC],
                    in_=yb)

            k_key = {}      # (pair, tcc) -> filler key (kr chunk ready after)
            q_key = {}      # (tcq, pair) -> filler key
            v_key = {}      # tt -> filler key
            TOT = NJ * NTC * NT       # global attention granules

            def g2pc(g):
                blk, sc = divmod(g, NT)
                tcq, pair = divmod(blk, NJ)
                return tcq, pair, sc

            p_store = {}
            avs_cur = [None]

            def emit_scores_g(g):
                tcq, pair, sc = g2pc(g)
                tsl = slice(tcq * TC, (tcq + 1) * TC)
                if sc == 0 and (tcq, pair) in q_key:
                    drain_key(q_key[(tcq, pair)])
                # drain one kr chunk ahead: the RoPE chain (DMA permute +
                # GpSimd/DVE elementwise) needs a few us after emission.
                tccs = (sc // 4,) if sc < 2 else (sc // 4,
                                                  min(sc // 4 + 1, NTC - 1))
                for tcc_d in tccs:
                    if (pair, tcc_d) in k_key:
                        drain_key(k_key[(pair, tcc_d)])
                pairP = psum.tile([128, 2 * TC], f32, tag="score",
                                  name=f"sc_{tcq}_{pair}_{sc}")
                for half in range(2):
                    p0 = 64 * half
                    nc.tensor.matmul(
                        pairP[:, half * TC:(half + 1) * TC],
                        lhsT=kr_sb[p0:p0 + 64, pair,
                                   sc * 128:(sc + 1) * 128],
                        rhs=qr_sb[p0:p0 + 64, pair, tsl],
                        tile_position=(p0, 0))
                p_sb = ppool.tile([128, 2 * TC], bf16, tag="p")
                nc.scalar.activation(
                    p_sb, pairP, mybir.ActivationFunctionType.Exp,
                    scale=float(1.0 / np.sqrt(D)))
                p_store[g] = p_sb

            def emit_normalize(tcq, pair, avs):
                # evacuate AV psum quickly (frees the banks), then batch the
                # two denominators through one [32,32] reciprocal.
                tsl = slice(tcq * TC, (tcq + 1) * TC)
                # one combined [65, 1024] evacuation tile so the whole DRAM
                # round-trip is 4 ordered SWDGE DMAs (gather, spread, store,
                # broadcast); the SWDGE descriptor ring drains strictly in
                # order, so the DRAM write->read pairs can never reorder
                # (the HWDGE queues round-robin and occasionally raced here).
                av_f = avsb.tile([D + 1, 2 * TC], f32, tag="avf")
                for half in range(2):
                    nc.vector.tensor_copy(
                        av_f[:, half * TC:(half + 1) * TC],
                        avs[half][0:D + 1, :])
                dn = rdram.tile([1, 2 * TC], f32, tag="dn")
                nc.gpsimd.dma_start(out=dn, in_=av_f[D:D + 1, :])
                d32 = rcp.tile([32, TC // 16], f32, tag="d32")
                nc.gpsimd.dma_start(
                    out=d32, in_=dn[:, :].rearrange("a (b c) -> (a b) c", c=TC // 16))
                r32 = rcp.tile([32, TC // 16], f32, tag="r32")
                nc.vector.reciprocal(r32, d32)
                rrd = rdram.tile([32, TC // 16], f32, tag="rrd")
                nc.gpsimd.dma_start(out=rrd, in_=r32)
                rrd_flat = rrd[:, :].rearrange("a b -> (a b)")
                r_bc = bass.AP(
                    tensor=rrd_flat.tensor, offset=rrd_flat.offset,
                    ap=[[0, D]] + [list(d) for d in rrd_flat.ap])
                rb = rbpool.tile([D, 2 * TC], f32, tag="rb")
                nc.gpsimd.dma_start(out=rb, in_=r_bc)
                for half in range(2):
                    p0 = 64 * half
                    nc.vector.tensor_mul(
                        ao_sb[p0:p0 + 64, pair, tsl],
                        av_f[0:D, half * TC:(half + 1) * TC],
                        rb[:, half * TC:(half + 1) * TC])

            def emit_av_g(g):
                tcq, pair, sc = g2pc(g)
                if sc == 0:
                    avs_cur[0] = []
                    for i in range(2):
                        av = psum.tile([D + 1, TC], f32, tag="av",
                                       name=f"av_{tcq}_{pair}_{i}")
                        # zero via DVE instead of relying on the first
                        # matmul's start=True bank-clear: the memset is a
                        # tracked write-after-read against the previous
                        # block's denominator-evac reads of this bank.
                        nc.vector.memset(av, 0.0)
                        avs_cur[0].append(av)
                avs = avs_cur[0]
                for half in range(2):
                    nc.tensor.matmul(
                        avs[half],
                        lhsT=v_sb[:, sc, 2 * pair + half, :],
                        rhs=p_store[g][:, half * TC:(half + 1) * TC],
                        start=False, stop=(sc == NT - 1))
                if sc == NT - 1:
                    emit_normalize(tcq, pair, avs)
                del p_store[g]

            # ================= schedule =================
            # Warm-ups: pull the ACT table load and the GpSimd ext-isa IRAM
            # load off the critical path.
            wtile = consts.tile([1, 16], f32)
            nc.vector.memset(wtile, 0.0)
            wtile2 = consts.tile([1, 16], f32)
            nc.scalar.activation(wtile2, wtile,
                                 mybir.ActivationFunctionType.Exp)
            wtile3 = consts.tile([1, 16], f32)
            nc.gpsimd.tensor_mul(wtile3, wtile, wtile2)

            # Keep the PE warm (HAM at 8/8) with dummy matmuls while the
            # first input DMAs land.
            dumw = consts.tile([128, 128], bf16)
            nc.gpsimd.memset(dumw, 0.0)
            dumin = consts.tile([128, TC], bf16)
            nc.vector.memset(dumin, 0.0)
            for i in range(44):
                pd = psum.tile([128, TC], f32, tag="work", name=f"warm_{i}")
                nc.tensor.matmul(pd, lhsT=dumw, rhs=dumin)

            # Pre-phase: first K chunk and first Q chunk, synchronously.
            run_gen(gen_proj("k", kr_sb, 0, 0, mm_perm=True))
            run_gen(gen_proj("q", qr_sb, 0, 0, mm_perm=True))
            k_key[(0, 0)] = 0
            q_key[(0, 0)] = 0

            # Fillers for pair 0 of tcq 0: V tiles + remaining K chunks,
            # deadline-ordered (v(tt) before AV granule tt, K(0,tcc) before
            # scores granule 4*tcc).
            v_order = [0, 1, None, 2, 3, None, 4, 5, None, 6, 7, 8]
            ktcc = 1
            for item in v_order:
                if item is None:
                    k_key[(0, ktcc)] = enqueue(
                        gen_proj("k", kr_sb, 0, ktcc, mm_perm=True))
                    ktcc += 1
                else:
                    v_key[item] = enqueue(gen_vproj(item))
            for tt in range(9, NT):
                v_key[tt] = enqueue(gen_vproj(tt))

            ypart_tiles = {}

            def gen_outproj_A(tt, u):
                """jc 0..2 of the last tcq's out-proj, run during block 15."""
                ps = psum.tile([128, TC], f32, tag="work", name=f"ypA_{tt}_{u}")
                for jc in range(2):
                    nc.tensor.matmul(
                        ps, lhsT=ao_sb[:, jc, tt * 128:(tt + 1) * 128],
                        rhs=wo_sb[:, jc, u * TC:(u + 1) * TC],
                        start=(jc == 0), stop=False)
                yield
                nc.tensor.matmul(
                    ps, lhsT=ao_sb[:, 2, tt * 128:(tt + 1) * 128],
                    rhs=wo_sb[:, 2, u * TC:(u + 1) * TC],
                    start=False, stop=True)
                yp = ypartp.tile([128, TC], f32, tag="ypart",
                                 name=f"ypart_{tt}_{u}")
                nc.vector.tensor_copy(yp, ps)
                ypart_tiles[(tt, u)] = yp

            def emit_outproj_B(tt, u):
                """jc 3 + combine + store, the only tail work left."""
                ps = psum.tile([128, TC], f32, tag="work", name=f"ypB_{tt}_{u}")
                nc.tensor.matmul(
                    ps, lhsT=ao_sb[:, NJ - 1, tt * 128:(tt + 1) * 128],
                    rhs=wo_sb[:, NJ - 1, u * TC:(u + 1) * TC],
                    start=True, stop=True)
                yb = ypool.tile([128, TC], bf16, tag="yb", name=f"ybB_{tt}_{u}")
                nc.vector.tensor_add(yb, ps, ypart_tiles[(tt, u)])
                nc.sync.dma_start(
                    out=y[tt * 128:(tt + 1) * 128, u * TC:(u + 1) * TC],
                    in_=yb)

            def on_block_start(blk):
                """Enqueue the prerequisites for block blk+1 and the lagged
                out-projection work."""
                tcq, pair = divmod(blk, NJ)
                nxt = blk + 1
                if nxt < NJ * NTC:
                    ntcq, npair = divmod(nxt, NJ)
                    if ntcq == 0:
                        k_key[(npair, 0)] = enqueue(
                            gen_proj("k", kr_sb, npair, 0, mm_perm=True))
                        q_key[(0, npair)] = enqueue(
                            gen_proj("q", qr_sb, npair, 0))
                        for tcc in range(1, NTC):
                            k_key[(npair, tcc)] = enqueue(
                                gen_proj("k", kr_sb, npair, tcc, mm_perm=True))
                    else:
                        q_key[(ntcq, npair)] = enqueue(
                            gen_proj("q", qr_sb, npair, ntcq))
                if tcq >= 1:
                    tt = (tcq - 1) * (TC // 128) + pair
                    enqueue(gen_outproj(tt, 0))
                    enqueue(gen_outproj(tt, 1))
                if blk == NJ * NTC - 1:
                    for tt in range((NTC - 1) * (TC // 128), NT):
                        for u in range(2):
                            enqueue(gen_outproj_A(tt, u))

            on_block_start(0)
            emit_scores_g(0)
            for g in range(TOT):
                if g + 1 < TOT:
                    if (g + 1) % NT == 0:
                        on_block_start((g + 1) // NT)
                    emit_scores_g(g + 1)
                tcq, pair, sc = g2pc(g)
                if tcq == 0 and pair == 0 and sc in v_key:
                    drain_key(v_key[sc])
                if sc == NT - 5:
                    blk_n = g // NT + 1
                    qk = q_key.get((blk_n // NJ, blk_n % NJ))
                    if qk is not None:
                        drain_key(qk)
                pump(3)
                emit_av_g(g)
            drain_all()
            for tt in range((NTC - 1) * (TC // 128), NT):
                emit_outproj_B(tt, 0)
                emit_outproj_B(tt, 1)

    return nc


_NC_CACHE = {}


def _get_nc(cfg, with_qk_bias):
    key = (cfg.T, cfg.CIN, cfg.JH, cfg.CO, cfg.D, with_qk_bias)
    if key not in _NC_CACHE:
        _NC_CACHE[key] = build_nc(cfg, with_qk_bias)
    return _NC_CACHE[key]


def make_in_maps(cfg, x, Wq, bq, Wk, bk, Wv, bv, Wo, bo, n_groups=2):
    """Build the per-core input dicts. Core c = b * n_groups + g."""
    bf = ml_dtypes.bfloat16
    B = x.shape[0]
    cos, sin = rope_tables(cfg)
    pm = perm_matrix()
    with_qk_bias = bool(np.any(bq) or np.any(bk))
    in_maps = []
    for b in range(B):
        for g in range(n_groups):
            rows = slice(g * cfg.JH, (g + 1) * cfg.JH)
            m = {
                "xT": np.ascontiguousarray(x[b].T).astype(bf),
                "wqT": np.ascontiguousarray(Wq[rows, :].T).astype(bf),
                "wkT": np.ascontiguousarray(Wk[rows, :].T).astype(bf),
                "wvT": np.ascontiguousarray(Wv[rows, :].T).astype(bf),
                "woT": np.ascontiguousarray(Wo[:, rows].T).astype(bf),
                "cosT": cos.astype(bf), "sinT": sin.astype(bf), "permM": pm,
            }
            if with_qk_bias:
                m["bq"] = np.ascontiguousarray(
                    bq[rows].reshape(cfg.NJ, 128).astype(np.float32))
                m["bk"] = np.ascontiguousarray(
                    bk[rows].reshape(cfg.NJ, 128).astype(np.float32))
            in_maps.append(m)
    return in_maps, with_qk_bias


def run(x, Wq, bq, Wk, bk, Wv, bv, Wo, bo, trace=False):
    from concourse.bass_utils import run_bass_kernel_spmd

    B, T, C = x.shape
    n_groups = 2
    cfg = Cfg(T=T, CIN=C, JH=C // n_groups, CO=C, D=64)
    in_maps, with_qk_bias = make_in_maps(
        cfg, x, Wq, bq, Wk, bk, Wv, bv, Wo, bo, n_groups)
    nc = _get_nc(cfg, with_qk_bias)
    res = run_bass_kernel_spmd(
        nc, in_maps, core_ids=list(range(len(in_maps))), trace=trace)
    out = np.zeros((B, T, C), dtype=np.float32)
    for c, r in enumerate(res.results):
        out[c // n_groups] += np.asarray(r["y"], dtype=np.float32)
    # linear bias corrections (exact): v-bias passes through softmax row-sum=1;
    # out-proj bias is additive.
    out += (bv.astype(np.float32) @ Wo.T.astype(np.float32) + bo.astype(np.float32))
    return out, res


def kernel(x, Wq, bq, Wk, bk, Wv, bv, Wo, bo):
    out, _ = run(
        np.asarray(x, dtype=np.float32),
        np.asarray(Wq, dtype=np.float32), np.asarray(bq, dtype=np.float32),
        np.asarray(Wk, dtype=np.float32), np.asarray(bk, dtype=np.float32),
        np.asarray(Wv, dtype=np.float32), np.asarray(bv, dtype=np.float32),
        np.asarray(Wo, dtype=np.float32), np.asarray(bo, dtype=np.float32))
    return out
